# revision 19
# baseline (speedup 1.0000x reference)
"""Attention-pooling kernel for Trainium2 (8 NeuronCores, SPMD data-parallel).

Computes, for x: [B, S, H] and w: [H, 1]:
    scores[b, s] = sum_h tanh(x[b, s, h]) * w[h]
    attn = softmax(scores, axis=s)
    out[b, h]   = sum_s attn[b, s] * x[b, s, h]

Sharding: data-parallel over batch B across 8 cores (32 batches/core),
w replicated. No inter-core communication; host concatenates the shards.

Redesign vs the 263 us baseline (DVE was 87% busy, GPSIMD 80%): now
~204-221 us, DMA-bound (HBM ~358 GB/s/core, 64 MB/core => ~190 us floor).

  * Score pipeline in mixed precision (rel-err 8e-3 vs 2e-2 budget):
    ACT writes tanh as bf16, the w-multiply is a DVE 2x-mode bf16 TT,
    and the h-reduction is L1 add (bf16, 2x) + L2 add (fp16 out - its
    10-bit mantissa makes the rounding negligible) + one 1x-mode
    fp32-out tensor_reduce over the remaining 32. This replaces two
    full fp32 1x reduce passes: DVE ~229 us -> ~180 us.
    NOTE fp16 TT measured 1x on DVE (no packed uop); only bf16 gets 2x.
  * NO GPSIMD: it shares an SBUF port with DVE, and any gpsimd op
    overlapping a DVE tensor_tensor degraded that TT ~5x (690 ns ->
    3480 ns measured). At a 6.2 us/batch DMA period the overlap is
    unavoidable, so offloading the multiply was net-negative.
  * Sliding-window context matmuls: matmul j uses lhsT=ebuf[:, j],
    rhs=xb[:, j:j+2, :] (xb padded to 33 tiles; window 31 reads the
    garbage pad whose products land in the never-read right half), so
    EVERY tile's useful product accumulates into cols 0:128 of ONE
    PSUM bank: the even/odd PSUM split and the hb-copy + sum_row fold
    of the baseline epilogue disappear (PSUM itself does the fold),
    while keeping the fp32r fast path's moving free size of 256.
  * tot matmul (rowsum.T @ ones) issues BEFORE the 33 ctx matmuls so
    reciprocal(b-3) never stalls the DVE queue on PE's round.
  * Output rows collected 4-per-group in a [1, 4H] tile; one 2 KB
    store DMA per group on the scalar HWDGE ring (a sync-ring wait
    would stall the SP sequencer and the x-load queue behind it).
  * Engine-queue order per round: tanh(b) leads ACT (it unblocks the
    moment x-load b lands), then exp(b-1) and orow(b-3) fill the round
    with long-ready work. stage4 is emitted one batch late so exp(b)
    sits after tanh(b+1); PSUM-reading ops trail by 3 batches.

Per-core budget at the 6.2 us/batch DMA pace: ACT ~4.9 us (tanh 3.7 +
exp/orow), DVE ~5.6 us (mul 2.3 + L1 1.2 + L2 0.7 + reduce 1.2 +
recip), PE ~4.8 us (34 matmuls), DMA 6.15 us (2 MB @ ~342 GB/s).
Softmax normalization is algebraically factored out of the weighted sum
(exp without max-subtraction is safe: |scores| < ~60 here).
"""

import numpy as np

import concourse.bass as bass
import concourse.tile as tile
from concourse import bacc, mybir
from concourse.bass_utils import run_bass_kernel_spmd

B, S, H = 256, 4096, 128
N_CORES = 8
B_SHARD = B // N_CORES  # 32
P = 128                 # SBUF partitions; also H
S_TILES = S // P        # 32  (s = p * S_TILES + t)
Q = 4                   # batches per PSUM/epilogue group
N_GROUPS = B_SHARD // Q

F32 = mybir.dt.float32
F32R = mybir.dt.float32r
F16 = mybir.dt.float16
BF16 = mybir.dt.bfloat16

# s-tiles [0, GS) of the score multiply run on GPSIMD, [GS, S_TILES) on
# DVE. GPSIMD measured ~3.3 cyc/elem on fp32 TT; fp16 assumed similar.
GS = 12

_nc_cache = None


def _build() -> bass.Bass:
    nc = bacc.Bacc(None, target_bir_lowering=False, enable_partition_id=False)

    x_ext = nc.declare_dram_parameter(
        "encoder_outputs", [B_SHARD, S, H], F32, isOutput=False
    )
    w_ext = nc.declare_dram_parameter(
        "attention_weights", [H, 1], F32, isOutput=False
    )
    out_ext = nc.declare_dram_parameter("out", [B_SHARD, H], F32, isOutput=True)

    gs = max(1, min(GS, S_TILES - 1))
    vs = S_TILES - gs

    with tile.TileContext(nc) as tc:
        with (
            tc.tile_pool(name="singles", bufs=1) as singles,
            tc.tile_pool(name="xpool", bufs=4) as xpool,
            tc.tile_pool(name="zpool", bufs=3) as zpool,
            tc.tile_pool(name="zpool2", bufs=3) as zpool2,
            tc.tile_pool(name="small", bufs=8) as small,
            tc.tile_pool(name="gsmall", bufs=3) as gsmall,
            tc.tile_pool(name="psum_ctx", bufs=4, space="PSUM") as psum_ctx_pool,
            tc.tile_pool(name="psum_tot", bufs=3, space="PSUM") as psum_tot_pool,
        ):
            # w broadcast across partitions: w_bcast[p, h] = w[h]
            w_bcast = singles.tile([P, H], F32)
            w_flat = w_ext[:].rearrange("h one -> (one h)")
            w_row = bass.AP(
                tensor=w_flat.tensor,
                offset=w_flat.offset,
                ap=[[0, P], w_flat.ap[0]],
            )
            nc.sync.dma_start(out=w_bcast, in_=w_row)

            ones_col = singles.tile([P, 1], F32)
            nc.vector.memset(ones_col, 1.0)

            # w replicated along the tile axis in bf16 for the DVE mul
            w_rep = singles.tile([P, S_TILES, H], BF16)
            for t in range(S_TILES):
                nc.vector.tensor_copy(w_rep[:, t, :], w_bcast)

            # pair view of DRAM: one 4 MB transfer per 2 batches; each
            # partition reads 2x 16 KB contiguous (256 descriptors/xfer)
            xv2 = x_ext[:].rearrange(
                "(pr two) (p t) h -> pr p two t h", two=2, p=P
            )

            st = [dict() for _ in range(B_SHARD)]
            gst = [dict() for _ in range(N_GROUPS)]

            def stage0(b, d):  # load one 4 MB pair (even b only)
                xb2 = xpool.tile(
                    [P, 2 * S_TILES + 1, H], F32R, tag="xb2", name="xb2"
                )
                nc.sync.dma_start(
                    out=xb2[:, 0 : 2 * S_TILES, :],
                    in_=xv2[b // 2].bitcast(F32R),
                )
                st[b]["xb"] = st[b + 1]["xb"] = xb2
                st[b]["base"] = 0
                st[b + 1]["base"] = S_TILES

            def stage1(b, d):  # tanh -> bf16 (one instr: ends sooner)
                xbf = d["xb"].bitcast(F32)
                base = d["base"]
                d["z"] = z = zpool.tile([P, S_TILES, H], BF16, tag="z", name="z")
                nc.scalar.activation(
                    out=z,
                    in_=xbf[:, base : base + S_TILES, :],
                    func=mybir.ActivationFunctionType.Tanh,
                )

            def stage2(b, d):  # score multiply, in place on DVE (2x mode)
                z = d["z"]
                nc.vector.tensor_mul(z, z, w_rep)

            def stage3(b, d):  # h-reduction: L1/L2 adds (2x) + fp32 reduce
                z = d["z"]
                nc.vector.tensor_add(
                    z[:, :, 0:64], z[:, :, 0:64], z[:, :, 64:128]
                )
                # L2 writes fp16 (10-bit mantissa: rounding negligible vs
                # the bf16 levels) so the tail reduce sees finer values
                z2 = zpool2.tile([P, S_TILES, 32], F16, tag="z2", name="z2")
                nc.vector.tensor_add(z2, z[:, :, 0:32], z[:, :, 32:64])
                scores = small.tile([P, S_TILES], F32, tag="scores")
                nc.vector.tensor_reduce(
                    out=scores,
                    in_=z2,
                    axis=mybir.AxisListType.X,
                    op=mybir.AluOpType.add,
                )
                d["scores"] = scores

            def stage4(b, d):  # exp + sliding-window fp32r matmuls
                q = b % Q
                if q == 0:
                    gst[b // Q]["orow4"] = gsmall.tile([1, Q * H], F32, tag="orow4", name="orow4")
                d["ebuf"] = ebuf = small.tile(
                    [P, S_TILES], F32R, tag="ebuf", name="ebuf"
                )
                d["rowsum"] = rowsum = small.tile([P, 1], F32, tag="rowsum", name="rowsum")
                nc.scalar.activation(
                    out=ebuf,
                    in_=d["scores"],
                    func=mybir.ActivationFunctionType.Exp,
                    accum_out=rowsum,
                )
                # Window j covers s-tiles [j, j+1]: the useful product
                # e_j*x_j always lands in PSUM cols 0:128, so every tile
                # accumulates into ONE bank and no even/odd fold is
                # needed. Window 31 reads the garbage 33rd tile; its
                # products land in the never-read right half. The fp32r
                # fast path keeps its moving free size of 256.
                # tot FIRST: recip(b) in a later DVE round waits on it,
                # so it must not sit behind the 33 ctx matmuls on PE
                tot_ps = psum_tot_pool.tile([1, 1], F32, name="tot_ps")
                nc.tensor.matmul(
                    tot_ps, rowsum, ones_col, start=True, stop=True
                )
                xb, base = d["xb"], d["base"]
                ps = psum_ctx_pool.tile([1, 2 * H], F32, tag="ps", name="ps")
                for j in range(S_TILES):
                    nc.tensor.matmul(
                        ps,
                        ebuf[:, j : j + 1],
                        xb[:, base + j : base + j + 2, :],
                        start=(j == 0),
                        stop=(j == S_TILES - 1),
                    )
                d["ps"], d["tot_ps"] = ps, tot_ps

            def stage5(b, d):  # normalize + store (group-batched DMA)
                q = b % Q
                orow4 = gst[b // Q]["orow4"]
                recip = small.tile([1, 1], F32, tag="recip")
                nc.vector.reciprocal(out=recip, in_=d["tot_ps"])
                # ctx sits directly in ps[0, 0:128]; normalize on ACT
                # (reads PSUM at base partition 0)
                nc.scalar.activation(
                    out=orow4[0:1, q * H : (q + 1) * H],
                    in_=d["ps"][0:1, 0:H],
                    func=mybir.ActivationFunctionType.Copy,
                    scale=recip,
                )
                if q == Q - 1:
                    # Scalar-ring HWDGE: a sync-ring wait here would stall
                    # the SP sequencer and the x-load queue behind it.
                    g0 = (b // Q) * Q
                    nc.scalar.dma_start(
                        out=out_ext[g0 : g0 + Q, :].rearrange("b h -> (b h)"),
                        in_=orow4[0:1, :],
                    )

            # stage4(b) is emitted one batch late so exp(b) sits AFTER
            # tanh(b+1) in ACT's in-order stream: by then scores(b) are
            # ready and ACT never stalls on the DVE score chain.
            # stage5(b-3) leads each iteration so its tiny recip/scale
            # ops sit FIRST in the DVE/ACT queues for that round.
            # Intra-round engine-queue order matters: tanh(b) leads the
            # ACT round (it unblocks the moment the x-load lands), then
            # exp(b-1) and orow(b-3) fill the rest of the round with
            # long-ready work. recip(b-3) trails the DVE round; its
            # tot_ps was computed at the head of PE's round b-2.
            for b in range(B_SHARD):
                if b % 2 == 0:
                    stage0(b, st[b])
                for stage in (stage1, stage2, stage3):
                    stage(b, st[b])
                if b >= 1:
                    stage4(b - 1, st[b - 1])
                if b >= 3:
                    stage5(b - 3, st[b - 3])
            stage4(B_SHARD - 1, st[B_SHARD - 1])
            for tail in (3, 2, 1):
                stage5(B_SHARD - tail, st[B_SHARD - tail])

    # Bacc pipeline: splits multi-sem waits (HW allows one per instr),
    # inserts GPSIMD library loads + ACT table loads, lowers extended ISA.
    nc.compile()
    return nc


def _get_nc() -> bass.Bass:
    global _nc_cache
    if _nc_cache is None:
        _nc_cache = _build()
    return _nc_cache


def run(encoder_outputs: np.ndarray, attention_weights: np.ndarray, **spmd_kwargs):
    """Run the SPMD kernel; returns (output [B, H], BassKernelResults)."""
    nc = _get_nc()
    x = np.ascontiguousarray(encoder_outputs, dtype=np.float32)
    w = np.ascontiguousarray(attention_weights, dtype=np.float32)
    assert x.shape == (B, S, H), x.shape
    assert w.shape == (H, 1), w.shape
    in_maps = [
        {
            "encoder_outputs": x[i * B_SHARD : (i + 1) * B_SHARD],
            "attention_weights": w,
        }
        for i in range(N_CORES)
    ]
    res = run_bass_kernel_spmd(nc, in_maps, core_ids=list(range(N_CORES)), **spmd_kwargs)
    out = np.concatenate(
        [res.results[i]["out"] for i in range(N_CORES)], axis=0
    ).astype(np.float32)
    return out, res


def kernel(encoder_outputs: np.ndarray, attention_weights: np.ndarray) -> np.ndarray:
    out, _ = run(encoder_outputs, attention_weights)
    return out


# revision 20
# speedup vs baseline: 1.1528x; 1.1528x over previous
"""Attention-pooling kernel for Trainium2 (8 NeuronCores, SPMD data-parallel).

Computes, for x: [B, S, H] and w: [H, 1]:
    scores[b, s] = sum_h tanh(x[b, s, h]) * w[h]
    attn = softmax(scores, axis=s)
    out[b, h]   = sum_s attn[b, s] * x[b, s, h]

Sharding: data-parallel over batch B across 8 cores (32 batches/core),
w replicated. No inter-core communication; host concatenates the shards.

Redesign vs the 263 us baseline (DVE was 87% busy, GPSIMD 80%): now
~204-221 us, DMA-bound (HBM ~358 GB/s/core, 64 MB/core => ~190 us floor).

  * Score pipeline in mixed precision (rel-err 8e-3 vs 2e-2 budget):
    ACT writes tanh as bf16, the w-multiply is a DVE 2x-mode bf16 TT,
    and the h-reduction is L1 add (bf16, 2x) + L2 add (fp16 out - its
    10-bit mantissa makes the rounding negligible) + one 1x-mode
    fp32-out tensor_reduce over the remaining 32. This replaces two
    full fp32 1x reduce passes: DVE ~229 us -> ~180 us.
    NOTE fp16 TT measured 1x on DVE (no packed uop); only bf16 gets 2x.
  * NO GPSIMD: it shares an SBUF port with DVE, and any gpsimd op
    overlapping a DVE tensor_tensor degraded that TT ~5x (690 ns ->
    3480 ns measured). At a 6.2 us/batch DMA period the overlap is
    unavoidable, so offloading the multiply was net-negative.
  * Sliding-window context matmuls: matmul j uses lhsT=ebuf[:, j],
    rhs=xb[:, j:j+2, :] (xb padded to 33 tiles; window 31 reads the
    garbage pad whose products land in the never-read right half), so
    EVERY tile's useful product accumulates into cols 0:128 of ONE
    PSUM bank: the even/odd PSUM split and the hb-copy + sum_row fold
    of the baseline epilogue disappear (PSUM itself does the fold),
    while keeping the fp32r fast path's moving free size of 256.
  * tot matmul (rowsum.T @ ones) issues BEFORE the 33 ctx matmuls so
    reciprocal(b-3) never stalls the DVE queue on PE's round.
  * Output rows collected 4-per-group in a [1, 4H] tile; one 2 KB
    store DMA per group on the scalar HWDGE ring (a sync-ring wait
    would stall the SP sequencer and the x-load queue behind it).
  * Engine-queue order per round: tanh(b) leads ACT (it unblocks the
    moment x-load b lands), then exp(b-1) and orow(b-3) fill the round
    with long-ready work. stage4 is emitted one batch late so exp(b)
    sits after tanh(b+1); PSUM-reading ops trail by 3 batches.

Per-core budget at the 6.2 us/batch DMA pace: ACT ~4.9 us (tanh 3.7 +
exp/orow), DVE ~5.6 us (mul 2.3 + L1 1.2 + L2 0.7 + reduce 1.2 +
recip), PE ~4.8 us (34 matmuls), DMA 6.15 us (2 MB @ ~342 GB/s).
Softmax normalization is algebraically factored out of the weighted sum
(exp without max-subtraction is safe: |scores| < ~60 here).
"""

import numpy as np

import concourse.bass as bass
import concourse.tile as tile
from concourse import bacc, mybir
from concourse.bass_utils import run_bass_kernel_spmd

B, S, H = 256, 4096, 128
N_CORES = 8
B_SHARD = B // N_CORES  # 32
P = 128                 # SBUF partitions; also H
S_TILES = S // P        # 32  (s = p * S_TILES + t)
Q = 4                   # batches per PSUM/epilogue group
N_GROUPS = B_SHARD // Q

F32 = mybir.dt.float32
F32R = mybir.dt.float32r
F16 = mybir.dt.float16
BF16 = mybir.dt.bfloat16

# s-tiles [0, GS) of the score multiply run on GPSIMD, [GS, S_TILES) on
# DVE. GPSIMD measured ~3.3 cyc/elem on fp32 TT; fp16 assumed similar.
GS = 12

_nc_cache = None


def _build() -> bass.Bass:
    nc = bacc.Bacc(None, target_bir_lowering=False, enable_partition_id=False)

    x_ext = nc.declare_dram_parameter(
        "encoder_outputs", [B_SHARD, S, H], F32, isOutput=False
    )
    w_ext = nc.declare_dram_parameter(
        "attention_weights", [H, 1], F32, isOutput=False
    )
    out_ext = nc.declare_dram_parameter("out", [B_SHARD, H], F32, isOutput=True)

    gs = max(1, min(GS, S_TILES - 1))
    vs = S_TILES - gs

    with tile.TileContext(nc) as tc:
        with (
            tc.tile_pool(name="singles", bufs=1) as singles,
            tc.tile_pool(name="xpool", bufs=9) as xpool,
            tc.tile_pool(name="zpool", bufs=3) as zpool,
            tc.tile_pool(name="zpool2", bufs=3) as zpool2,
            tc.tile_pool(name="small", bufs=8) as small,
            tc.tile_pool(name="gsmall", bufs=3) as gsmall,
            tc.tile_pool(name="psum_ctx", bufs=4, space="PSUM") as psum_ctx_pool,
            tc.tile_pool(name="psum_tot", bufs=3, space="PSUM") as psum_tot_pool,
        ):
            # w broadcast across partitions: w_bcast[p, h] = w[h]
            w_bcast = singles.tile([P, H], F32)
            w_flat = w_ext[:].rearrange("h one -> (one h)")
            w_row = bass.AP(
                tensor=w_flat.tensor,
                offset=w_flat.offset,
                ap=[[0, P], w_flat.ap[0]],
            )
            nc.sync.dma_start(out=w_bcast, in_=w_row)

            ones_col = singles.tile([P, 1], F32)
            nc.vector.memset(ones_col, 1.0)

            # w replicated along the tile axis in bf16 for the DVE mul
            w_rep = singles.tile([P, S_TILES, H], BF16)
            for t in range(S_TILES):
                nc.vector.tensor_copy(w_rep[:, t, :], w_bcast)

            # [b, p, t, h] view of DRAM; partition p reads 16 KB contiguous
            xv = x_ext[:].rearrange("b (p t) h -> b p t h", p=P)

            st = [dict() for _ in range(B_SHARD)]
            gst = [dict() for _ in range(N_GROUPS)]

            def stage0(b, d):  # load (33rd tile left as garbage pad)
                d["xb"] = xb = xpool.tile(
                    [P, S_TILES + 1, H], F32R, tag="xb", name="xb"
                )
                nc.sync.dma_start(
                    out=xb[:, 0:S_TILES, :], in_=xv[b].bitcast(F32R)
                )

            def stage1(b, d):  # tanh -> bf16 (one instr: ends sooner)
                xbf = d["xb"].bitcast(F32)
                d["z"] = z = zpool.tile([P, S_TILES, H], BF16, tag="z", name="z")
                nc.scalar.activation(
                    out=z,
                    in_=xbf[:, 0:S_TILES, :],
                    func=mybir.ActivationFunctionType.Tanh,
                )

            def stage2(b, d):  # score multiply, in place on DVE (2x mode)
                z = d["z"]
                nc.vector.tensor_mul(z, z, w_rep)

            def stage3(b, d):  # h-reduction: L1/L2 adds (2x) + fp32 reduce
                z = d["z"]
                nc.vector.tensor_add(
                    z[:, :, 0:64], z[:, :, 0:64], z[:, :, 64:128]
                )
                # L2 writes fp16 (10-bit mantissa: rounding negligible vs
                # the bf16 levels) so the tail reduce sees finer values
                z2 = zpool2.tile([P, S_TILES, 32], F16, tag="z2", name="z2")
                nc.vector.tensor_add(z2, z[:, :, 0:32], z[:, :, 32:64])
                scores = small.tile([P, S_TILES], F32, tag="scores")
                nc.vector.tensor_reduce(
                    out=scores,
                    in_=z2,
                    axis=mybir.AxisListType.X,
                    op=mybir.AluOpType.add,
                )
                d["scores"] = scores

            def stage4(b, d):  # exp + sliding-window fp32r matmuls
                q = b % Q
                if q == 0:
                    gst[b // Q]["orow4"] = gsmall.tile([1, Q * H], F32, tag="orow4", name="orow4")
                d["ebuf"] = ebuf = small.tile(
                    [P, S_TILES], F32R, tag="ebuf", name="ebuf"
                )
                d["rowsum"] = rowsum = small.tile([P, 1], F32, tag="rowsum", name="rowsum")
                nc.scalar.activation(
                    out=ebuf,
                    in_=d["scores"],
                    func=mybir.ActivationFunctionType.Exp,
                    accum_out=rowsum,
                )
                # Window j covers s-tiles [j, j+1]: the useful product
                # e_j*x_j always lands in PSUM cols 0:128, so every tile
                # accumulates into ONE bank and no even/odd fold is
                # needed. Window 31 reads the garbage 33rd tile; its
                # products land in the never-read right half. The fp32r
                # fast path keeps its moving free size of 256.
                # tot FIRST: recip(b) in a later DVE round waits on it,
                # so it must not sit behind the 33 ctx matmuls on PE
                tot_ps = psum_tot_pool.tile([1, 1], F32, name="tot_ps")
                nc.tensor.matmul(
                    tot_ps, rowsum, ones_col, start=True, stop=True
                )
                xb = d["xb"]
                ps = psum_ctx_pool.tile([1, 2 * H], F32, tag="ps", name="ps")
                for j in range(S_TILES):
                    nc.tensor.matmul(
                        ps,
                        ebuf[:, j : j + 1],
                        xb[:, j : j + 2, :],
                        start=(j == 0),
                        stop=(j == S_TILES - 1),
                    )
                d["ps"], d["tot_ps"] = ps, tot_ps

            def stage5(b, d):  # normalize + store (group-batched DMA)
                q = b % Q
                orow4 = gst[b // Q]["orow4"]
                recip = small.tile([1, 1], F32, tag="recip")
                nc.vector.reciprocal(out=recip, in_=d["tot_ps"])
                # ctx sits directly in ps[0, 0:128]; normalize on ACT
                # (reads PSUM at base partition 0)
                nc.scalar.activation(
                    out=orow4[0:1, q * H : (q + 1) * H],
                    in_=d["ps"][0:1, 0:H],
                    func=mybir.ActivationFunctionType.Copy,
                    scale=recip,
                )
                if q == Q - 1:
                    # Scalar-ring HWDGE: a sync-ring wait here would stall
                    # the SP sequencer and the x-load queue behind it.
                    g0 = (b // Q) * Q
                    nc.scalar.dma_start(
                        out=out_ext[g0 : g0 + Q, :].rearrange("b h -> (b h)"),
                        in_=orow4[0:1, :],
                    )

            # stage4(b) is emitted one batch late so exp(b) sits AFTER
            # tanh(b+1) in ACT's in-order stream: by then scores(b) are
            # ready and ACT never stalls on the DVE score chain.
            # stage5(b-3) leads each iteration so its tiny recip/scale
            # ops sit FIRST in the DVE/ACT queues for that round.
            # Intra-round engine-queue order matters: tanh(b) leads the
            # ACT round (it unblocks the moment the x-load lands), then
            # exp(b-1) and orow(b-3) fill the rest of the round with
            # long-ready work. recip(b-3) trails the DVE round; its
            # tot_ps was computed at the head of PE's round b-2.
            for b in range(B_SHARD):
                for stage in (stage0, stage1, stage2, stage3):
                    stage(b, st[b])
                if b >= 1:
                    stage4(b - 1, st[b - 1])
                if b >= 3:
                    stage5(b - 3, st[b - 3])
            stage4(B_SHARD - 1, st[B_SHARD - 1])
            for tail in (3, 2, 1):
                stage5(B_SHARD - tail, st[B_SHARD - tail])

    # Bacc pipeline: splits multi-sem waits (HW allows one per instr),
    # inserts GPSIMD library loads + ACT table loads, lowers extended ISA.
    nc.compile()
    return nc


def _get_nc() -> bass.Bass:
    global _nc_cache
    if _nc_cache is None:
        _nc_cache = _build()
    return _nc_cache


def run(encoder_outputs: np.ndarray, attention_weights: np.ndarray, **spmd_kwargs):
    """Run the SPMD kernel; returns (output [B, H], BassKernelResults)."""
    nc = _get_nc()
    x = np.ascontiguousarray(encoder_outputs, dtype=np.float32)
    w = np.ascontiguousarray(attention_weights, dtype=np.float32)
    assert x.shape == (B, S, H), x.shape
    assert w.shape == (H, 1), w.shape
    in_maps = [
        {
            "encoder_outputs": x[i * B_SHARD : (i + 1) * B_SHARD],
            "attention_weights": w,
        }
        for i in range(N_CORES)
    ]
    res = run_bass_kernel_spmd(nc, in_maps, core_ids=list(range(N_CORES)), **spmd_kwargs)
    out = np.concatenate(
        [res.results[i]["out"] for i in range(N_CORES)], axis=0
    ).astype(np.float32)
    return out, res


def kernel(encoder_outputs: np.ndarray, attention_weights: np.ndarray) -> np.ndarray:
    out, _ = run(encoder_outputs, attention_weights)
    return out
